# revision 26
# baseline (speedup 1.0000x reference)
"""Trainium2 Bass kernel for nn_MixedAttentionModule (CvT-style mixed attention block).

Data-parallel over batch: 32 batches -> 8 cores x 4 batches. No collectives.
Channel-major layouts on device (activations [C, n]); host pre-transposes x and
folds LN/BN/bias affines into adjacent weights. Depthwise 3x3 convs run on the
tensor engine as 9 diagonal matmuls accumulating in PSUM.

v2 schedule: software-pipelined attention (scores of head-pair j+1 overlap the
sum/attn@V matmuls of j; softmax inverse on the DVE via reciprocal_approx_fast
instead of ACT Ln/Exp), FFN with ph2(ft-1) interleaved after ph1(ft) to hide the
gelu latency, LN1 stats of batch b+1 colocated with LN2(b) so the apply overlaps
the FFN, residual taken from the bf16 x copy (no f32 x stream), and startup
weight DMAs moved to the scalar HWDGE queue.
"""
import sys

sys.path.insert(0, "/opt/trn_rl_repo")

import numpy as np
import ml_dtypes

B, n, C, NH, HD, FF = 32, 1024, 768, 12, 64, 3072
Ht = Wt = 32
M = 256          # kv positions (16*16)
NCORES = 8
BL = B // NCORES  # batches per core
EPS = 1e-5
KT = C // 128     # 6 channel tiles
FT = FF // 128    # 24 ff tiles
F32 = None
BF16 = None

_BUILD_CACHE = {}


def _patch_compiler(ldw_opt=True):
    """Patch bass' walrus invocation: keep the standard pass list but allow
    toggling the LDWEIGHTS-dedup codegen optimization."""
    from pathlib import Path
    from concourse import bass_utils

    def patched(tmpdir, inp="bir.json", outp="file.neff", arch=None, *, dve_root=None):
        cmd = [
            bass_utils.get_walrus_driver(),
            "--pass",
            "birverifier,runtime_memory_reservation,lower_act,lower_dve,"
            "lower_ap_offset,codegen,neff_packager",
            "-i", inp,
            "--neff-output-filename", outp,
            "--enable-birsim=true",
            "--mem-mode=physical",
            "--policy=0",
            f"--enable-ldw-opt={'true' if ldw_opt else 'false'}",
            "--assign-static-dmas-to-sp=false",
            f"--dram-page-size={bass_utils.aot_getenv('NEURON_SCRATCHPAD_PAGE_SIZE', '256')}",
            "--enable-neff-debug-info=true",
            "--jobs", "8",
            *bass_utils.get_walrus_args(
                bass_utils.get_bir_arch(tmpdir, inp) if arch is None else arch,
                tmpdir, dve_root=dve_root,
            ),
        ]
        result = bass_utils.run_command(cmd, cwd=tmpdir)
        if result is not None:
            (Path(tmpdir) / "log.txt").write_text(result.stdout)
        return f"{tmpdir}/{outp}"

    bass_utils.bir_verify_and_optimise = patched


def _split_sync_waits(nc, max_waits=1):
    """walrus codegen in this environment allows at most one sync wait per
    instruction. Hoist excess waits onto standalone EventSemaphore carriers
    inserted just before, on the same engine (engines execute their stream
    in order, so this is equivalent)."""
    from concourse import mybir

    n_new = 0
    for f in nc.m.functions:
        for blk in f.blocks:
            out = []
            for inst in blk.instructions:
                si = getattr(inst, "sync_info", None)
                if si is not None:
                    waits = list(si.on_wait or [])
                    ups = list(si.on_update or [])
                    if len(waits) > max_waits:
                        extra = waits[: len(waits) - max_waits]
                        keep = waits[len(waits) - max_waits:]
                        for w in extra:
                            n_new += 1
                            out.append(mybir.InstEventSemaphore(
                                name=f"syncw-{n_new}-{inst.name}",
                                ins=[], outs=[],
                                engine=inst.engine,
                                sync_info=mybir.SyncInfo(on_wait=[w], on_update=[]),
                            ))
                        inst.sync_info = mybir.SyncInfo(on_wait=keep, on_update=ups)
                out.append(inst)
            blk.instructions = out
    return n_new


def _build_program():
    from concourse import bass, mybir, tile

    f32 = mybir.dt.float32
    bf16 = mybir.dt.bfloat16
    Alu = mybir.AluOpType
    Act = mybir.ActivationFunctionType

    nc = bass.Bass("TRN2", target_bir_lowering=False, debug=False, num_devices=NCORES)

    # ---- DRAM I/O ----
    xTb = nc.dram_tensor("xTb", [BL, C, n], bf16, kind="ExternalInput").ap()
    wqT = nc.dram_tensor("wqT", [C, C], bf16, kind="ExternalInput").ap()
    wkT = nc.dram_tensor("wkT", [C, C], bf16, kind="ExternalInput").ap()
    wvT = nc.dram_tensor("wvT", [C, C], bf16, kind="ExternalInput").ap()
    w1p = nc.dram_tensor("w1p", [FT, 128, C], bf16, kind="ExternalInput").ap()
    w2T = nc.dram_tensor("w2T", [FF, C], bf16, kind="ExternalInput").ap()
    dq9 = nc.dram_tensor("dq9", [KT, 128, 9 * 128], bf16, kind="ExternalInput").ap()
    dk9 = nc.dram_tensor("dk9", [KT, 128, 9 * 128], bf16, kind="ExternalInput").ap()
    dv9 = nc.dram_tensor("dv9", [KT, 128, 9 * 128], bf16, kind="ExternalInput").ap()
    bq_d = nc.dram_tensor("bq", [128, KT], f32, kind="ExternalInput").ap()
    bk_d = nc.dram_tensor("bk", [128, KT], f32, kind="ExternalInput").ap()
    bva_d = nc.dram_tensor("bva", [128, KT], f32, kind="ExternalInput").ap()
    b1_d = nc.dram_tensor("b1", [128, FT], f32, kind="ExternalInput").ap()
    ones_sq_d = nc.dram_tensor("ones_sq", [128, 128], bf16, kind="ExternalInput").ap()
    outT = nc.dram_tensor("outT", [BL, C, n], f32, kind="ExternalOutput").ap()

    with tile.TileContext(nc) as tc:
        with tc.tile_pool(name="P", bufs=1) as P:
            # ---- persistent SBUF ----
            wq_sb = [P.tile([128, C], bf16, name=f"wq{k}", tag="wq", bufs=KT) for k in range(KT)]
            wk_sb = [P.tile([128, C], bf16, name=f"wk{k}", tag="wk", bufs=KT) for k in range(KT)]
            wv_sb = [P.tile([128, C], bf16, name=f"wv{k}", tag="wv", bufs=KT) for k in range(KT)]
            bqp = P.tile([128, KT], f32, name="bqp", tag="bqp", bufs=1)
            bkp = P.tile([128, KT], f32, name="bkp", tag="bkp", bufs=1)
            bvap = P.tile([128, KT], f32, name="bvap", tag="bvap", bufs=1)
            b1p = P.tile([128, FT], f32, name="b1p", tag="b1p", bufs=1)
            bq_sb = [bqp[:, k:k + 1] for k in range(KT)]
            bk_sb = [bkp[:, k:k + 1] for k in range(KT)]
            bva_sb = [bvap[:, k:k + 1] for k in range(KT)]
            b1_sb = [b1p[:, t:t + 1] for t in range(FT)]
            ones_sq = P.tile([128, 128], bf16, name="onessq", tag="onessq", bufs=1)

            pad = [P.tile([128, 34, 34], bf16, name=f"pad{k}", tag="pad", bufs=KT) for k in range(KT)]
            xb_sb = [P.tile([128, n], bf16, name=f"xb{k}", tag="xb", bufs=KT) for k in range(KT)]
            actkv = [P.tile([128, 2 * M], bf16, name=f"akv{k}", tag="akv", bufs=KT) for k in range(KT)]
            qT = [P.tile([128, n], bf16, name=f"qT{k}", tag="qT", bufs=KT) for k in range(KT)]
            kTt = [P.tile([128, M], bf16, name=f"kT{k}", tag="kT", bufs=KT) for k in range(KT)]
            vtok = [P.tile([128, C], bf16, name=f"vt{k}", tag="vt", bufs=2) for k in range(2)]
            x2b = [P.tile([128, n], bf16, name=f"x2{k}", tag="x2", bufs=KT) for k in range(KT)]

            # DMA order at startup: x (sync HWDGE) before anything else; the
            # big projection weights go on the scalar HWDGE queue so they
            # don't gate the first batch's LN/conv.
            nc.sync.dma_start(ones_sq[:], ones_sq_d[:, :])
            for k in range(KT):
                nc.sync.dma_start(xb_sb[k][:], xTb[0, k * 128:(k + 1) * 128, :])
            nc.sync.dma_start(bqp[:], bq_d[:, :])
            nc.sync.dma_start(bkp[:], bk_d[:, :])
            nc.sync.dma_start(bvap[:], bva_d[:, :])
            nc.sync.dma_start(b1p[:], b1_d[:, :])
            for k in range(KT):
                nc.sync.dma_start(wq_sb[k][:], wqT[k * 128:(k + 1) * 128, :])
                nc.sync.dma_start(wk_sb[k][:], wkT[k * 128:(k + 1) * 128, :])
                nc.sync.dma_start(wv_sb[k][:], wvT[k * 128:(k + 1) * 128, :])

            # zero the padded conv buffers once (interiors are overwritten per
            # batch; the one-element borders must stay zero)
            for k in range(KT):
                nc.vector.memset(pad[k][:], 0.0)

            # constant APs for float biases of activation ops
            czero = P.tile([128, 1], f32, name="czero", tag="cz", bufs=2)
            nc.vector.memset(czero[:], 0.0)
            nc.const_aps.aps[(f32, 0.0)] = czero[:]
            ceps = P.tile([128, 1], f32, name="ceps", tag="cz", bufs=2)
            nc.vector.memset(ceps[:], EPS)
            nc.const_aps.aps[(f32, EPS)] = ceps[:]

            def ln_stats(psp, src_tiles, label):
                """Channel-dim LN stats via ones-matmuls (results broadcast
                across all 128 partitions). Returns (mbc, rbc) [128, n] f32."""
                mean = psp.tile([128, n], f32, name=f"mean{label}", tag="stat", bufs=4)
                msq = psp.tile([128, n], f32, name=f"msq{label}", tag="stat", bufs=4)
                for k in range(KT):
                    sqt = P.tile([128, n], bf16, name=f"sq{label}{k}", tag="sq", bufs=2)
                    # squares on DVE: the surrounding region is ACT-bound
                    # (attention exps + softmax inverse), DVE has slack
                    nc.vector.tensor_mul(sqt[:], src_tiles[k][:], src_tiles[k][:])
                    for ch in range(2):
                        sl = slice(ch * 512, (ch + 1) * 512)
                        nc.tensor.matmul(mean[:, sl], ones_sq[:], src_tiles[k][:, sl],
                                         start=(k == 0), stop=(k == KT - 1))
                        nc.tensor.matmul(msq[:, sl], ones_sq[:], sqt[:, sl],
                                         start=(k == 0), stop=(k == KT - 1))
                mbc = P.tile([128, n], bf16, name=f"mbc{label}", tag="mbc", bufs=2)
                rbc = P.tile([128, n], bf16, name=f"rbc{label}", tag="rbc", bufs=2)
                nc.vector.tensor_scalar_mul(mbc[:], mean[:], 1.0 / C)
                nc.vector.tensor_mul(rbc[:], mbc[:], mbc[:])
                nc.vector.scalar_tensor_tensor(rbc[:], msq[:], 1.0 / C,
                                               rbc[:], Alu.mult, Alu.subtract)
                # rstd = exp(-0.5*ln(var+eps)) on ACT (same table set as Exp)
                nc.scalar.activation(rbc[:], rbc[:], Act.Ln, bias=EPS)
                nc.scalar.activation(rbc[:], rbc[:], Act.Exp, scale=-0.5)
                return mbc, rbc

            def ln_apply_pad(src_tiles, mbc, rbc, label):
                """(src-m)*rstd written into the padded conv interiors."""
                for k in range(KT):
                    tmp = P.tile([128, n], bf16, name=f"tp{label}{k}", tag="tfv", bufs=2)
                    nc.vector.tensor_sub(tmp[:], src_tiles[k][:], mbc[:])
                    nc.vector.tensor_mul(pad[k][:, 1:33, 1:33],
                                         tmp[:].rearrange("p (a c) -> p a c", a=32),
                                         rbc[:].rearrange("p (a c) -> p a c", a=32))

            def ln_apply_flat(src_tiles, mbc, rbc, dst_tiles, label):
                """ch-major (FFN ch0 can start after the first half) and
                DVE/GpSimd alternation per k."""
                for ch in range(2):
                    sl = slice(ch * 512, (ch + 1) * 512)
                    for k in range(KT):
                        tmp = P.tile([128, 512], bf16, name=f"tl{label}{k}_{ch}",
                                     tag="tfv", bufs=2, padded_shape=[128, n])
                        nc.vector.tensor_sub(tmp[:], src_tiles[k][:, sl], mbc[:, sl])
                        nc.vector.tensor_mul(dst_tiles[k][:, sl], tmp[:], rbc[:, sl])

            def elu(ps_ap, dst_ap, width, label):
                """elu+1 = relu(x) + exp(min(x,0)); the -1 is folded into the
                projection biases on the host."""
                tmin = P.tile([128, width], f32, name=f"tm{label}", tag="tfv",
                              bufs=2, padded_shape=[128, n])
                et = P.tile([128, width], bf16, name=f"ee{label}", tag="ee",
                            bufs=2, padded_shape=[128, n])
                nc.vector.tensor_scalar_min(tmin[:], ps_ap, 0.0)
                nc.scalar.activation(et[:], tmin[:], Act.Exp)
                nc.vector.scalar_tensor_tensor(dst_ap, ps_ap, 0.0, et[:], Alu.max, Alu.add)

            # ---------------- prologue: LN1 of batch 0 ----------------
            with tc.tile_pool(name="ps_pre", bufs=1, space="PSUM") as psp:
                mbc, rbc = ln_stats(psp, xb_sb, "p0")
                ln_apply_pad(xb_sb, mbc, rbc, "p0")

            for b in range(BL):
                # ============ conv + projections ============
                with tc.tile_pool(name=f"ps_cv{b}", bufs=1, space="PSUM") as cvp:
                    actq = []
                    for k in range(KT):
                        dqt = P.tile([128, 9 * 128], bf16, name=f"dq{b}_{k}", tag="dq", bufs=3)
                        nc.gpsimd.dma_start(dqt[:], dq9[k, :, :])
                        cq = cvp.tile([128, n], f32, name=f"cq{b}_{k}", tag="cq", bufs=2)
                        for tap in range(9):
                            dy, dx = tap // 3, tap % 3
                            for ch in range(2):
                                rhs = pad[k][:, dy + 16 * ch:dy + 16 * ch + 16, dx:dx + 32]
                                nc.tensor.matmul(cq[:, ch * 512:(ch + 1) * 512],
                                                 dqt[:, tap * 128:(tap + 1) * 128], rhs,
                                                 start=(tap == 0), stop=(tap == 8))
                        aq = P.tile([128, n], bf16, name=f"aq{b}_{k}", tag="a10", bufs=KT)
                        actq.append(aq)
                        elu(cq[:], aq[:], n, f"q{b}_{k}")

                        dkt = P.tile([128, 9 * 128], bf16, name=f"dk{b}_{k}", tag="dkv", bufs=4)
                        dvt = P.tile([128, 9 * 128], bf16, name=f"dv{b}_{k}", tag="dkv", bufs=4)
                        nc.gpsimd.dma_start(dkt[:], dk9[k, :, :])
                        nc.gpsimd.dma_start(dvt[:], dv9[k, :, :])
                        ckv = cvp.tile([128, 2 * M], f32, name=f"ckv{b}_{k}", tag="ckv", bufs=2)
                        rhs2 = [pad[k][:, dy:dy + 32:2, dx:dx + 32:2]
                                for dy, dx in [(t // 3, t % 3) for t in range(9)]]
                        for tap in range(9):
                            nc.tensor.matmul(ckv[:, 0:M], dkt[:, tap * 128:(tap + 1) * 128],
                                             rhs2[tap], start=(tap == 0), stop=(tap == 8))
                        for tap in range(9):
                            nc.tensor.matmul(ckv[:, M:2 * M], dvt[:, tap * 128:(tap + 1) * 128],
                                             rhs2[tap], start=(tap == 0), stop=(tap == 8))
                        elu(ckv[:], actkv[k][:], 2 * M, f"kv{b}_{k}")

                    # projections (both n-chunks per stationary for reuse)
                    for mt in range(KT):
                        pq = cvp.tile([128, n], f32, name=f"pq{b}_{mt}", tag="cq", bufs=2)
                        for k in range(KT):
                            for ch in range(2):
                                sl = slice(ch * 512, (ch + 1) * 512)
                                nc.tensor.matmul(pq[:, sl], wq_sb[k][:, mt * 128:(mt + 1) * 128],
                                                 actq[k][:, sl],
                                                 start=(k == 0), stop=(k == KT - 1))
                        nc.scalar.activation(qT[mt][:], pq[:], Act.Identity, bias=bq_sb[mt][:])
                    for mt in range(KT):
                        pk = cvp.tile([128, 2 * M], f32, name=f"pk{b}_{mt}", tag="ckv", bufs=2)
                        for k in range(KT):
                            nc.tensor.matmul(pk[:, 0:M], wk_sb[k][:, mt * 128:(mt + 1) * 128],
                                             actkv[k][:, 0:M],
                                             start=(k == 0), stop=(k == KT - 1))
                        nc.scalar.activation(kTt[mt][:], pk[:, 0:M], Act.Identity, bias=bk_sb[mt][:])
                    for mt2 in range(2):
                        pv = cvp.tile([128, n], f32, name=f"pv{b}_{mt2}", tag="cq", bufs=2)
                        for k in range(KT):
                            for ch, w in [(0, 512), (1, 256)]:
                                nc.tensor.matmul(pv[:, ch * 512:ch * 512 + w],
                                                 actkv[k][:, M + mt2 * 128:M + (mt2 + 1) * 128],
                                                 wv_sb[k][:, ch * 512:ch * 512 + w],
                                                 start=(k == 0), stop=(k == KT - 1))
                        nc.vector.tensor_copy(vtok[mt2][:, :], pv[:, 0:C])

                # ============ attention (j-pipelined) ============
                with tc.tile_pool(name=f"ps_at{b}", bufs=1, space="PSUM") as atp:
                    ET = {}     # ET[j] -> list of 4 tiles in unit order
                    sbc = {}
                    sinv = {}
                    po = {}
                    UNITS = [(0, 0), (1, 0), (0, 1), (1, 1)]  # (hh, mt)

                    def s_unit(j, u):
                        hh, mt = UNITS[u]
                        bp = 64 * hh
                        ps = atp.tile([128, n], f32, name=f"sc{b}_{j}_{u}", tag="smm", bufs=2)
                        for ch in range(2):
                            sl = slice(ch * 512, (ch + 1) * 512)
                            nc.tensor.matmul(ps[:, sl],
                                             kTt[j][bp:bp + 64, mt * 128:(mt + 1) * 128],
                                             qT[j][bp:bp + 64, sl],
                                             tile_position=(bp, 0))
                        et = P.tile([128, n], bf16, name=f"ET{b}_{j}_{u}", tag="ET", bufs=8)
                        nc.scalar.activation(et[:], ps[:], Act.Exp, scale=0.125)
                        ET.setdefault(j, [None] * 4)[u] = et

                    def u_pair(j, u):
                        # u indexes (hh, ch); accumulate over mt into sbc[j]
                        hh, ch = u // 2, u % 2
                        bp = 64 * hh
                        sl = slice(ch * 512, (ch + 1) * 512)
                        if u == 0:
                            sbc[j] = atp.tile([128, n], f32, name=f"sb{b}_{j}", tag="sbc", bufs=1)
                        ets = ET[j]
                        for mt in range(2):
                            et = ets[{(0, 0): 0, (1, 0): 1, (0, 1): 2, (1, 1): 3}[(hh, mt)]]
                            nc.tensor.matmul(sbc[j][bp:bp + 64, sl], ones_sq[:, 0:64],
                                             et[:, sl], tile_position=(0, bp),
                                             start=(mt == 0), stop=(mt == 1))

                    def u_recip(j):
                        # 1/s = exp(-ln(s)) on ACT (same table set as the exps)
                        sinv[j] = P.tile([128, n], f32, name=f"si{b}_{j}", tag="sinv", bufs=2)
                        nc.scalar.activation(sinv[j][:], sbc[j][:], Act.Ln)
                        nc.scalar.activation(sinv[j][:], sinv[j][:], Act.Exp, scale=-1.0)

                    def v_pair(j, u):
                        hh, ch = u // 2, u % 2
                        bp = 64 * hh
                        h = 2 * j + hh
                        sl = slice(ch * 512, (ch + 1) * 512)
                        if u == 0:
                            po[j] = atp.tile([128, n], f32, name=f"po{b}_{j}", tag="po", bufs=1)
                        ets = ET[j]
                        for mt in range(2):
                            et = ets[{(0, 0): 0, (1, 0): 1, (0, 1): 2, (1, 1): 3}[(hh, mt)]]
                            nc.tensor.matmul(po[j][bp:bp + 64, sl],
                                             vtok[mt][:, h * 64:(h + 1) * 64],
                                             et[:, sl], tile_position=(0, bp),
                                             start=(mt == 0), stop=(mt == 1))

                    def v_tail(j):
                        ot = P.tile([128, n], bf16, name=f"ot{b}_{j}", tag="ot", bufs=2)
                        nc.vector.tensor_mul(ot[:], po[j][:], sinv[j][:])
                        # residual: x2 = (attn_out + bva) + x   (bf16 x copy)
                        nc.vector.scalar_tensor_tensor(x2b[j][:], ot[:], bva_sb[j][:],
                                                       xb_sb[j][:], Alu.add, Alu.add)

                    for u in range(4):
                        s_unit(0, u)
                    for j in range(NH // 2):
                        nxt = j + 1
                        if nxt < NH // 2:
                            for u in range(4):
                                s_unit(nxt, u)
                                u_pair(j, u)
                        else:
                            for u in range(4):
                                u_pair(j, u)
                        u_recip(j)
                        for u in range(4):
                            v_pair(j, u)
                        v_tail(j)

                # next batch's x: emitted after the last batch-b reader of xb
                # (the attention residual), lands during LN2(b) stats
                if b + 1 < BL:
                    for k in range(KT):
                        nc.sync.dma_start(xb_sb[k][:], xTb[b + 1, k * 128:(k + 1) * 128, :])

                # ============ LN2(b) + LN1(b+1) stats ============
                xl2 = [P.tile([128, n], bf16, name=f"xl2{b}_{k}", tag="a10", bufs=KT)
                       for k in range(KT)]
                with tc.tile_pool(name=f"ps_st{b}", bufs=1, space="PSUM") as stp:
                    mbc2, rbc2 = ln_stats(stp, x2b, f"c{b}")
                    ln_apply_flat(x2b, mbc2, rbc2, xl2, f"c{b}")
                    if b + 1 < BL:
                        mbc1, rbc1 = ln_stats(stp, xb_sb, f"a{b + 1}")
                        ln_apply_pad(xb_sb, mbc1, rbc1, f"a{b + 1}")

                # ============ FFN + final residual ============
                with tc.tile_pool(name=f"ps_ffn{b}", bufs=1, space="PSUM") as ffp:
                    for ch in range(2):
                        sl = slice(ch * 512, (ch + 1) * 512)
                        ph2 = [ffp.tile([128, 512], f32, name=f"h2_{b}_{ch}_{mt}", tag="h2", bufs=6)
                               for mt in range(KT)]
                        prev = None
                        for ft in range(FT):
                            w1b = P.tile([128, C], bf16, name=f"w1_{b}_{ch}_{ft}", tag="w1", bufs=4)
                            nc.gpsimd.dma_start(w1b[:], w1p[ft, :, :])
                            w2b = P.tile([128, C], bf16, name=f"w2_{b}_{ch}_{ft}", tag="w2", bufs=4)
                            nc.gpsimd.dma_start(w2b[:], w2T[ft * 128:(ft + 1) * 128, :])
                            ph1 = ffp.tile([128, 512], f32, name=f"h1_{b}_{ch}_{ft}", tag="h1", bufs=2)
                            for k in range(KT):
                                nc.tensor.matmul(ph1[:], w1b[:, k * 128:(k + 1) * 128],
                                                 xl2[k][:, sl],
                                                 start=(k == 0), stop=(k == KT - 1))
                            gt = P.tile([128, 512], bf16, name=f"g_{b}_{ch}_{ft}", tag="g", bufs=3)
                            nc.scalar.activation(gt[:], ph1[:], Act.Gelu, bias=b1_sb[ft][:])
                            if prev is not None:
                                pw2, pgt, pft = prev
                                for mt in range(KT):
                                    nc.tensor.matmul(ph2[mt][:], pw2[:, mt * 128:(mt + 1) * 128],
                                                     pgt[:], start=(pft == 0), stop=(pft == FT - 1))
                            prev = (w2b, gt, ft)
                        pw2, pgt, pft = prev
                        for mt in range(KT):
                            nc.tensor.matmul(ph2[mt][:], pw2[:, mt * 128:(mt + 1) * 128],
                                             pgt[:], start=(pft == 0), stop=(pft == FT - 1))
                        for mt in range(KT):
                            ob = P.tile([128, 512], f32, name=f"o_{b}_{ch}_{mt}", tag="ob", bufs=3)
                            nc.vector.tensor_add(ob[:], x2b[mt][:, sl], ph2[mt][:])
                            nc.sync.dma_start(outT[b, mt * 128:(mt + 1) * 128, sl], ob[:])

    n_hoisted = _split_sync_waits(nc)
    print(f"_split_sync_waits: hoisted waits onto {n_hoisted} carrier instructions")
    return nc


def _host_prep(inputs):
    """Fold LN/BN affines into weights; build packed bf16 arrays."""
    f = lambda k: np.asarray(inputs[k], np.float32)
    bfc = lambda a: np.ascontiguousarray(a.astype(ml_dtypes.bfloat16))
    x = f("x")                         # (B, n, C)
    ln1_g, ln1_b = f("ln1_g"), f("ln1_b")
    ln2_g, ln2_b = f("ln2_g"), f("ln2_b")

    prep = {}
    xT = np.ascontiguousarray(x.transpose(0, 2, 1))   # (B, C, n)
    prep["xTb"] = bfc(xT)

    diag9 = {}
    badj = {}
    for nm in ["q", "k", "v"]:
        w = f(f"dw_w_{nm}")[:, 0]                     # (C,3,3)
        w_eff = w * ln1_g[:, None, None]
        cb = f(f"dw_b_{nm}") + ln1_b * w.sum((1, 2))  # exact only if ln1_b == 0 (boundary)
        assert np.abs(cb).max() < 1e-30, "nonzero conv bias not implemented on device"
        sc = f(f"bn_g_{nm}") / np.sqrt(f(f"bn_v_{nm}") + EPS)
        sh = f(f"bn_b_{nm}") - f(f"bn_m_{nm}") * sc
        W = f(f"W_{nm}")
        W_eff = W * sc[None, :]
        # device computes elu+1 (the -1 is folded here); also BN shift
        b_eff = f(f"b_{nm}") + W @ sh - W_eff.sum(1)
        # pack 9 taps of diagonal matrices: [KT, 128, 9*128]
        d = np.zeros((KT, 128, 9 * 128), np.float32)
        for kt in range(KT):
            ww = w_eff[kt * 128:(kt + 1) * 128]       # (128,3,3)
            for tap in range(9):
                dy, dx = tap // 3, tap % 3
                d[kt, np.arange(128), tap * 128 + np.arange(128)] = ww[:, dy, dx]
        diag9[nm] = bfc(d)
        badj[nm] = b_eff
        prep[f"w{nm}T"] = bfc(np.ascontiguousarray(W_eff.T))
    prep["dq9"], prep["dk9"], prep["dv9"] = diag9["q"], diag9["k"], diag9["v"]
    prep["bq"] = np.ascontiguousarray(badj["q"].reshape(KT, 128).T)
    prep["bk"] = np.ascontiguousarray(badj["k"].reshape(KT, 128).T)
    prep["bva"] = np.ascontiguousarray(badj["v"].reshape(KT, 128).T)

    W1 = f("W1") * ln2_g[None, :]                     # (FF, C)
    b1 = f("b1") + f("W1") @ ln2_b
    W2 = f("W2")                                      # (C, FF)
    assert np.abs(f("b2")).max() < 1e-30, "nonzero b2 not implemented on device"
    W1T = W1.T                                        # (C, FF) = [cin, f]
    w1pk = np.zeros((FT, 128, C), np.float32)         # [ft, cin_p, kt*128+f]
    for ft in range(FT):
        blk = W1T[:, ft * 128:(ft + 1) * 128]         # (C, 128)
        w1pk[ft] = blk.reshape(KT, 128, 128).transpose(1, 0, 2).reshape(128, C)
    prep["w1p"] = bfc(w1pk)
    prep["w2T"] = bfc(np.ascontiguousarray(W2.T))     # (FF, C)
    prep["b1"] = np.ascontiguousarray(b1.reshape(FT, 128).T)
    prep["ones_sq"] = np.ones((128, 128), ml_dtypes.bfloat16)
    return prep


def kernel(**inputs):
    from concourse.bass_utils import run_bass_kernel_spmd

    _patch_compiler(ldw_opt=_BUILD_CACHE.get("ldw_opt", False))
    if "nc" not in _BUILD_CACHE:
        _BUILD_CACHE["nc"] = _build_program()
    nc = _BUILD_CACHE["nc"]

    prep = _host_prep(inputs)
    shared = {k: v for k, v in prep.items() if k != "xTb"}
    in_maps = []
    for c in range(NCORES):
        im = dict(shared)
        im["xTb"] = np.ascontiguousarray(prep["xTb"][c * BL:(c + 1) * BL])
        in_maps.append(im)

    res = run_bass_kernel_spmd(nc, in_maps, list(range(NCORES)),
                               **_BUILD_CACHE.get("run_kwargs", {}))
    _BUILD_CACHE["last_results"] = res
    outs = [res.results[c]["outT"].transpose(0, 2, 1) for c in range(NCORES)]
    return np.ascontiguousarray(np.concatenate(outs, 0).astype(np.float32))


# revision 32
# speedup vs baseline: 1.2238x; 1.2238x over previous
"""Trainium2 Bass kernel for nn_MixedAttentionModule (CvT-style mixed attention block).

Data-parallel over batch: 32 batches -> 8 cores x 4 batches. No collectives.
Channel-major layouts on device (activations [C, n]); host pre-transposes x and
folds LN/BN/bias affines into adjacent weights. Depthwise 3x3 convs run on the
tensor engine as 9 diagonal matmuls accumulating in PSUM.

v2 schedule: software-pipelined attention (scores of head-pair j+1 overlap the
sum/attn@V matmuls of j; softmax inverse on the DVE via reciprocal_approx_fast
instead of ACT Ln/Exp), FFN with ph2(ft-1) interleaved after ph1(ft) to hide the
gelu latency, LN1 stats of batch b+1 colocated with LN2(b) so the apply overlaps
the FFN, residual taken from the bf16 x copy (no f32 x stream), and startup
weight DMAs moved to the scalar HWDGE queue.
"""
import sys

sys.path.insert(0, "/opt/trn_rl_repo")

import numpy as np
import ml_dtypes

B, n, C, NH, HD, FF = 32, 1024, 768, 12, 64, 3072
Ht = Wt = 32
M = 256          # kv positions (16*16)
NCORES = 8
BL = B // NCORES  # batches per core
EPS = 1e-5
KT = C // 128     # 6 channel tiles
FT = FF // 128    # 24 ff tiles
F32 = None
BF16 = None

_BUILD_CACHE = {}


def _patch_compiler(ldw_opt=True):
    """Patch bass' walrus invocation: keep the standard pass list but allow
    toggling the LDWEIGHTS-dedup codegen optimization."""
    from pathlib import Path
    from concourse import bass_utils

    def patched(tmpdir, inp="bir.json", outp="file.neff", arch=None, *, dve_root=None):
        cmd = [
            bass_utils.get_walrus_driver(),
            "--pass",
            "birverifier,runtime_memory_reservation,lower_act,lower_dve,"
            "lower_ap_offset,codegen,neff_packager",
            "-i", inp,
            "--neff-output-filename", outp,
            "--enable-birsim=true",
            "--mem-mode=physical",
            "--policy=0",
            f"--enable-ldw-opt={'true' if ldw_opt else 'false'}",
            "--assign-static-dmas-to-sp=false",
            f"--dram-page-size={bass_utils.aot_getenv('NEURON_SCRATCHPAD_PAGE_SIZE', '256')}",
            "--enable-neff-debug-info=true",
            "--jobs", "8",
            *bass_utils.get_walrus_args(
                bass_utils.get_bir_arch(tmpdir, inp) if arch is None else arch,
                tmpdir, dve_root=dve_root,
            ),
        ]
        result = bass_utils.run_command(cmd, cwd=tmpdir)
        if result is not None:
            (Path(tmpdir) / "log.txt").write_text(result.stdout)
        return f"{tmpdir}/{outp}"

    bass_utils.bir_verify_and_optimise = patched


def _split_sync_waits(nc, max_waits=1):
    """walrus codegen in this environment allows at most one sync wait per
    instruction. Hoist excess waits onto standalone EventSemaphore carriers
    inserted just before, on the same engine (engines execute their stream
    in order, so this is equivalent)."""
    from concourse import mybir

    n_new = 0
    for f in nc.m.functions:
        for blk in f.blocks:
            out = []
            for inst in blk.instructions:
                si = getattr(inst, "sync_info", None)
                if si is not None:
                    waits = list(si.on_wait or [])
                    ups = list(si.on_update or [])
                    if len(waits) > max_waits:
                        extra = waits[: len(waits) - max_waits]
                        keep = waits[len(waits) - max_waits:]
                        for w in extra:
                            n_new += 1
                            out.append(mybir.InstEventSemaphore(
                                name=f"syncw-{n_new}-{inst.name}",
                                ins=[], outs=[],
                                engine=inst.engine,
                                sync_info=mybir.SyncInfo(on_wait=[w], on_update=[]),
                            ))
                        inst.sync_info = mybir.SyncInfo(on_wait=keep, on_update=ups)
                out.append(inst)
            blk.instructions = out
    return n_new


def _build_program():
    from concourse import bass, mybir, tile

    f32 = mybir.dt.float32
    bf16 = mybir.dt.bfloat16
    Alu = mybir.AluOpType
    Act = mybir.ActivationFunctionType

    nc = bass.Bass("TRN2", target_bir_lowering=False, debug=False, num_devices=NCORES)

    # ---- DRAM I/O ----
    xTb = nc.dram_tensor("xTb", [BL, C, n], bf16, kind="ExternalInput").ap()
    wqT = nc.dram_tensor("wqT", [C, C], bf16, kind="ExternalInput").ap()
    wkT = nc.dram_tensor("wkT", [C, C], bf16, kind="ExternalInput").ap()
    wvT = nc.dram_tensor("wvT", [C, C], bf16, kind="ExternalInput").ap()
    w1p = nc.dram_tensor("w1p", [FT, 128, C], bf16, kind="ExternalInput").ap()
    w2T = nc.dram_tensor("w2T", [FF, C], bf16, kind="ExternalInput").ap()
    dq9 = nc.dram_tensor("dq9", [KT, 128, 9 * 128], bf16, kind="ExternalInput").ap()
    dk9 = nc.dram_tensor("dk9", [KT, 128, 9 * 128], bf16, kind="ExternalInput").ap()
    dv9 = nc.dram_tensor("dv9", [KT, 128, 9 * 128], bf16, kind="ExternalInput").ap()
    bq_d = nc.dram_tensor("bq", [128, KT], f32, kind="ExternalInput").ap()
    bk_d = nc.dram_tensor("bk", [128, KT], f32, kind="ExternalInput").ap()
    bva_d = nc.dram_tensor("bva", [128, KT], f32, kind="ExternalInput").ap()
    b1_d = nc.dram_tensor("b1", [128, FT], f32, kind="ExternalInput").ap()
    ones_sq_d = nc.dram_tensor("ones_sq", [128, 128], bf16, kind="ExternalInput").ap()
    outT = nc.dram_tensor("outT", [BL, C, n], f32, kind="ExternalOutput").ap()

    with tile.TileContext(nc) as tc:
        with tc.tile_pool(name="P", bufs=1) as P:
            # ---- persistent SBUF ----
            wq_sb = [P.tile([128, C], bf16, name=f"wq{k}", tag="wq", bufs=KT) for k in range(KT)]
            wk_sb = [P.tile([128, C], bf16, name=f"wk{k}", tag="wk", bufs=KT) for k in range(KT)]
            wv_sb = [P.tile([128, C], bf16, name=f"wv{k}", tag="wv", bufs=KT) for k in range(KT)]
            bqp = P.tile([128, KT], f32, name="bqp", tag="bqp", bufs=1)
            bkp = P.tile([128, KT], f32, name="bkp", tag="bkp", bufs=1)
            bvap = P.tile([128, KT], f32, name="bvap", tag="bvap", bufs=1)
            b1p = P.tile([128, FT], f32, name="b1p", tag="b1p", bufs=1)
            bq_sb = [bqp[:, k:k + 1] for k in range(KT)]
            bk_sb = [bkp[:, k:k + 1] for k in range(KT)]
            bva_sb = [bvap[:, k:k + 1] for k in range(KT)]
            b1_sb = [b1p[:, t:t + 1] for t in range(FT)]
            ones_sq = P.tile([128, 128], bf16, name="onessq", tag="onessq", bufs=1)

            pad = [P.tile([128, 34, 34], bf16, name=f"pad{k}", tag="pad", bufs=KT) for k in range(KT)]
            xb_sb = [P.tile([128, n], bf16, name=f"xb{k}", tag="xb", bufs=KT) for k in range(KT)]
            actkv = [P.tile([128, 2 * M], bf16, name=f"akv{k}", tag="akv", bufs=KT) for k in range(KT)]
            qT = [P.tile([128, n], bf16, name=f"qT{k}", tag="qT", bufs=KT) for k in range(KT)]
            kTt = [P.tile([128, M], bf16, name=f"kT{k}", tag="kT", bufs=KT) for k in range(KT)]
            vtok = [P.tile([128, C], bf16, name=f"vt{k}", tag="vt", bufs=2) for k in range(2)]
            x2b = [P.tile([128, n], bf16, name=f"x2{k}", tag="x2", bufs=KT) for k in range(KT)]

            # DMA order at startup: x (sync HWDGE) before anything else; the
            # big projection weights go on the scalar HWDGE queue so they
            # don't gate the first batch's LN/conv.
            nc.sync.dma_start(ones_sq[:], ones_sq_d[:, :])
            for k in range(KT):
                nc.sync.dma_start(xb_sb[k][:], xTb[0, k * 128:(k + 1) * 128, :])
            nc.sync.dma_start(bqp[:], bq_d[:, :])
            nc.sync.dma_start(bkp[:], bk_d[:, :])
            nc.sync.dma_start(bvap[:], bva_d[:, :])
            nc.sync.dma_start(b1p[:], b1_d[:, :])
            for k in range(KT):
                nc.sync.dma_start(wq_sb[k][:], wqT[k * 128:(k + 1) * 128, :])
                nc.sync.dma_start(wk_sb[k][:], wkT[k * 128:(k + 1) * 128, :])
                nc.sync.dma_start(wv_sb[k][:], wvT[k * 128:(k + 1) * 128, :])

            # zero the padded conv buffers once (interiors are overwritten per
            # batch; the one-element borders must stay zero)
            for k in range(KT):
                nc.vector.memset(pad[k][:], 0.0)

            # constant APs for float biases of activation ops
            czero = P.tile([128, 1], f32, name="czero", tag="cz", bufs=2)
            nc.vector.memset(czero[:], 0.0)
            nc.const_aps.aps[(f32, 0.0)] = czero[:]
            ceps = P.tile([128, 1], f32, name="ceps", tag="cz", bufs=2)
            nc.vector.memset(ceps[:], EPS)
            nc.const_aps.aps[(f32, EPS)] = ceps[:]

            def ln_stats(psp, src_tiles, label, tag="stat", bufs=4):
                """Channel-dim LN stats via ones-matmuls (results broadcast
                across all 128 partitions). Returns (mbc, rbc) [128, n] bf16."""
                mean = psp.tile([128, n], f32, name=f"mean{label}", tag=tag, bufs=bufs)
                msq = psp.tile([128, n], f32, name=f"msq{label}", tag=tag, bufs=bufs)
                for k in range(KT):
                    sqt = P.tile([128, n], bf16, name=f"sq{label}{k}", tag="sq", bufs=2)
                    nc.scalar.activation(sqt[:], src_tiles[k][:], Act.Square)
                    for ch in range(2):
                        sl = slice(ch * 512, (ch + 1) * 512)
                        nc.tensor.matmul(mean[:, sl], ones_sq[:], src_tiles[k][:, sl],
                                         start=(k == 0), stop=(k == KT - 1))
                        nc.tensor.matmul(msq[:, sl], ones_sq[:], sqt[:, sl],
                                         start=(k == 0), stop=(k == KT - 1))
                mbc = P.tile([128, n], bf16, name=f"mbc{label}", tag="mbc", bufs=2)
                rbc = P.tile([128, n], bf16, name=f"rbc{label}", tag="rbc", bufs=2)
                nc.vector.tensor_scalar_mul(mbc[:], mean[:], 1.0 / C)
                nc.vector.tensor_mul(rbc[:], mbc[:], mbc[:])
                nc.vector.scalar_tensor_tensor(rbc[:], msq[:], 1.0 / C,
                                               rbc[:], Alu.mult, Alu.subtract)
                # rstd = exp(-0.5*ln(var+eps)) on ACT (same table set as Exp)
                nc.scalar.activation(rbc[:], rbc[:], Act.Ln, bias=EPS)
                nc.scalar.activation(rbc[:], rbc[:], Act.Exp, scale=-0.5)
                return mbc, rbc

            def ln_apply_pad(src_tiles, mbc, rbc, label, engines=None):
                """(src-m)*rstd written into the padded conv interiors."""
                engines = engines or [nc.vector]
                for k in range(KT):
                    eng = engines[k % len(engines)]
                    tag = "tfv" if eng is nc.vector else "tfg"
                    tmp = P.tile([128, n], bf16, name=f"tp{label}{k}", tag=tag, bufs=2)
                    eng.tensor_sub(tmp[:], src_tiles[k][:], mbc[:])
                    eng.tensor_mul(pad[k][:, 1:33, 1:33],
                                   tmp[:].rearrange("p (a c) -> p a c", a=32),
                                   rbc[:].rearrange("p (a c) -> p a c", a=32))

            def ln_apply_flat(src_tiles, mbc, rbc, dst_tiles, label):
                """ch-major (FFN ch0 can start after the first half) and
                DVE/GpSimd alternation per k."""
                for ch in range(2):
                    sl = slice(ch * 512, (ch + 1) * 512)
                    for k in range(KT):
                        tmp = P.tile([128, 512], bf16, name=f"tl{label}{k}_{ch}",
                                     tag="tfv", bufs=2, padded_shape=[128, n])
                        nc.vector.tensor_sub(tmp[:], src_tiles[k][:, sl], mbc[:, sl])
                        nc.vector.tensor_mul(dst_tiles[k][:, sl], tmp[:], rbc[:, sl])

            def elu(ps_ap, dst_ap, width, label):
                """elu+1 = relu(x) + exp(min(x,0)); the -1 is folded into the
                projection biases on the host."""
                tmin = P.tile([128, width], f32, name=f"tm{label}", tag="tfv",
                              bufs=2, padded_shape=[128, n])
                et = P.tile([128, width], bf16, name=f"ee{label}", tag="ee",
                            bufs=2, padded_shape=[128, n])
                nc.vector.tensor_scalar_min(tmin[:], ps_ap, 0.0)
                nc.scalar.activation(et[:], tmin[:], Act.Exp)
                nc.vector.scalar_tensor_tensor(dst_ap, ps_ap, 0.0, et[:], Alu.max, Alu.add)

            # ---------------- prologue: LN1 of batch 0 ----------------
            with tc.tile_pool(name="ps_pre", bufs=1, space="PSUM") as psp:
                mbc, rbc = ln_stats(psp, xb_sb, "p0", bufs=2)
                ln_apply_pad(xb_sb, mbc, rbc, "p0", engines=[nc.vector, nc.gpsimd])

            for b in range(BL):
                # one PSUM pool for conv+proj+attention+stats: big [128,1024]
                # x3 slots (6 banks) + ckv [128,512] x2 (2 banks) — phase
                # transitions pipeline instead of serializing on pool WARs
                with tc.tile_pool(name=f"ps_ca{b}", bufs=1, space="PSUM") as cap:
                    # ============ conv + projections ============
                    actq = []
                    for k in range(KT):
                        dqt = P.tile([128, 9 * 128], bf16, name=f"dq{b}_{k}", tag="dq", bufs=3)
                        nc.gpsimd.dma_start(dqt[:], dq9[k, :, :])
                        cq = cap.tile([128, n], f32, name=f"cq{b}_{k}", tag="big", bufs=3)
                        for tap in range(9):
                            dy, dx = tap // 3, tap % 3
                            for ch in range(2):
                                rhs = pad[k][:, dy + 16 * ch:dy + 16 * ch + 16, dx:dx + 32]
                                nc.tensor.matmul(cq[:, ch * 512:(ch + 1) * 512],
                                                 dqt[:, tap * 128:(tap + 1) * 128], rhs,
                                                 start=(tap == 0), stop=(tap == 8))
                        aq = P.tile([128, n], bf16, name=f"aq{b}_{k}", tag="a10", bufs=KT)
                        actq.append(aq)
                        elu(cq[:], aq[:], n, f"q{b}_{k}")

                        dkt = P.tile([128, 9 * 128], bf16, name=f"dk{b}_{k}", tag="dkv", bufs=4)
                        dvt = P.tile([128, 9 * 128], bf16, name=f"dv{b}_{k}", tag="dkv", bufs=4)
                        nc.gpsimd.dma_start(dkt[:], dk9[k, :, :])
                        nc.gpsimd.dma_start(dvt[:], dv9[k, :, :])
                        ckv = cap.tile([128, 2 * M], f32, name=f"ckv{b}_{k}", tag="ckv", bufs=2)
                        rhs2 = [pad[k][:, dy:dy + 32:2, dx:dx + 32:2]
                                for dy, dx in [(t // 3, t % 3) for t in range(9)]]
                        for tap in range(9):
                            nc.tensor.matmul(ckv[:, 0:M], dkt[:, tap * 128:(tap + 1) * 128],
                                             rhs2[tap], start=(tap == 0), stop=(tap == 8))
                        for tap in range(9):
                            nc.tensor.matmul(ckv[:, M:2 * M], dvt[:, tap * 128:(tap + 1) * 128],
                                             rhs2[tap], start=(tap == 0), stop=(tap == 8))
                        elu(ckv[:], actkv[k][:], 2 * M, f"kv{b}_{k}")

                    # projections (both n-chunks per stationary for reuse)
                    for mt in range(KT):
                        pq = cap.tile([128, n], f32, name=f"pq{b}_{mt}", tag="big", bufs=3)
                        for k in range(KT):
                            for ch in range(2):
                                sl = slice(ch * 512, (ch + 1) * 512)
                                nc.tensor.matmul(pq[:, sl], wq_sb[k][:, mt * 128:(mt + 1) * 128],
                                                 actq[k][:, sl],
                                                 start=(k == 0), stop=(k == KT - 1))
                        nc.scalar.activation(qT[mt][:], pq[:], Act.Identity, bias=bq_sb[mt][:])
                    for mt in range(KT):
                        pk = cap.tile([128, 2 * M], f32, name=f"pk{b}_{mt}", tag="ckv", bufs=2)
                        for k in range(KT):
                            nc.tensor.matmul(pk[:, 0:M], wk_sb[k][:, mt * 128:(mt + 1) * 128],
                                             actkv[k][:, 0:M],
                                             start=(k == 0), stop=(k == KT - 1))
                        nc.scalar.activation(kTt[mt][:], pk[:, 0:M], Act.Identity, bias=bk_sb[mt][:])
                    for mt2 in range(2):
                        pv = cap.tile([128, n], f32, name=f"pv{b}_{mt2}", tag="big", bufs=3)
                        for k in range(KT):
                            for ch, w in [(0, 512), (1, 256)]:
                                nc.tensor.matmul(pv[:, ch * 512:ch * 512 + w],
                                                 actkv[k][:, M + mt2 * 128:M + (mt2 + 1) * 128],
                                                 wv_sb[k][:, ch * 512:ch * 512 + w],
                                                 start=(k == 0), stop=(k == KT - 1))
                        nc.vector.tensor_copy(vtok[mt2][:, :], pv[:, 0:C])

                    # ============ attention (j-pipelined) ============
                    ET = {}     # ET[j] -> list of 4 tiles in unit order
                    sbc = {}
                    sinv = {}
                    po = {}
                    UNITS = [(0, 0), (1, 0), (0, 1), (1, 1)]  # (hh, mt)

                    def s_unit(j, u):
                        hh, mt = UNITS[u]
                        bp = 64 * hh
                        ps = cap.tile([128, n], f32, name=f"sc{b}_{j}_{u}", tag="big", bufs=3)
                        for ch in range(2):
                            sl = slice(ch * 512, (ch + 1) * 512)
                            nc.tensor.matmul(ps[:, sl],
                                             kTt[j][bp:bp + 64, mt * 128:(mt + 1) * 128],
                                             qT[j][bp:bp + 64, sl],
                                             tile_position=(bp, 0))
                        et = P.tile([128, n], bf16, name=f"ET{b}_{j}_{u}", tag="ET", bufs=8)
                        nc.scalar.activation(et[:], ps[:], Act.Exp, scale=0.125)
                        ET.setdefault(j, [None] * 4)[u] = et

                    def u_pair(j, u):
                        # u indexes (hh, ch); accumulate over mt into sbc[j]
                        hh, ch = u // 2, u % 2
                        bp = 64 * hh
                        sl = slice(ch * 512, (ch + 1) * 512)
                        if u == 0:
                            sbc[j] = cap.tile([128, n], f32, name=f"sb{b}_{j}", tag="big", bufs=3)
                        ets = ET[j]
                        for mt in range(2):
                            et = ets[{(0, 0): 0, (1, 0): 1, (0, 1): 2, (1, 1): 3}[(hh, mt)]]
                            nc.tensor.matmul(sbc[j][bp:bp + 64, sl], ones_sq[:, 0:64],
                                             et[:, sl], tile_position=(0, bp),
                                             start=(mt == 0), stop=(mt == 1))

                    def u_recip(j):
                        # 1/s = exp(-ln(s)) on ACT (same table set as the exps)
                        sinv[j] = P.tile([128, n], f32, name=f"si{b}_{j}", tag="sinv", bufs=2)
                        nc.scalar.activation(sinv[j][:], sbc[j][:], Act.Ln)
                        nc.scalar.activation(sinv[j][:], sinv[j][:], Act.Exp, scale=-1.0)

                    def v_pair(j, u):
                        hh, ch = u // 2, u % 2
                        bp = 64 * hh
                        h = 2 * j + hh
                        sl = slice(ch * 512, (ch + 1) * 512)
                        if hh == 0:
                            po[(j, ch)] = cap.tile([128, 2 * M], f32, name=f"po{b}_{j}_{ch}",
                                                   tag="ckv", bufs=2)
                        ets = ET[j]
                        for mt in range(2):
                            et = ets[{(0, 0): 0, (1, 0): 1, (0, 1): 2, (1, 1): 3}[(hh, mt)]]
                            nc.tensor.matmul(po[(j, ch)][bp:bp + 64, :],
                                             vtok[mt][:, h * 64:(h + 1) * 64],
                                             et[:, sl], tile_position=(0, bp),
                                             start=(mt == 0), stop=(mt == 1))

                    def v_tail(j):
                        ot = P.tile([128, n], bf16, name=f"ot{b}_{j}", tag="ot", bufs=2)
                        for ch in range(2):
                            sl = slice(ch * 512, (ch + 1) * 512)
                            nc.vector.tensor_mul(ot[:, sl], po[(j, ch)][:], sinv[j][:, sl])
                        # residual: x2 = (attn_out + bva) + x   (bf16 x copy)
                        nc.vector.scalar_tensor_tensor(x2b[j][:], ot[:], bva_sb[j][:],
                                                       xb_sb[j][:], Alu.add, Alu.add)

                    for u in range(4):
                        s_unit(0, u)
                    for j in range(NH // 2):
                        nxt = j + 1
                        if nxt < NH // 2:
                            for u in range(3):
                                s_unit(nxt, u)
                                u_pair(j, u)
                            u_pair(j, 3)
                            u_recip(j)
                            s_unit(nxt, 3)
                        else:
                            for u in range(4):
                                u_pair(j, u)
                            u_recip(j)
                        for u in range(4):
                            v_pair(j, u)
                        v_tail(j)

                    # next batch's x: emitted after the last batch-b reader of
                    # xb (the attention residual), lands during LN2(b) stats
                    if b + 1 < BL:
                        for k in range(KT):
                            nc.sync.dma_start(xb_sb[k][:], xTb[b + 1, k * 128:(k + 1) * 128, :])

                    # ============ LN2(b) + LN1(b+1) stats ============
                    xl2 = [P.tile([128, n], bf16, name=f"xl2{b}_{k}", tag="a10", bufs=KT)
                           for k in range(KT)]
                    mbc2, rbc2 = ln_stats(cap, x2b, f"c{b}", tag="big", bufs=3)
                    ln_apply_flat(x2b, mbc2, rbc2, xl2, f"c{b}")
                    if b + 1 < BL:
                        mbc1, rbc1 = ln_stats(cap, xb_sb, f"a{b + 1}", tag="big", bufs=3)
                        ln_apply_pad(xb_sb, mbc1, rbc1, f"a{b + 1}")

                # ============ FFN + final residual ============
                with tc.tile_pool(name=f"ps_ffn{b}", bufs=1, space="PSUM") as ffp:
                    for ch in range(2):
                        sl = slice(ch * 512, (ch + 1) * 512)
                        ph2 = [ffp.tile([128, 512], f32, name=f"h2_{b}_{ch}_{mt}", tag="h2", bufs=6)
                               for mt in range(KT)]
                        prev = None
                        for ft in range(FT):
                            w1b = P.tile([128, C], bf16, name=f"w1_{b}_{ch}_{ft}", tag="w1", bufs=4)
                            nc.gpsimd.dma_start(w1b[:], w1p[ft, :, :])
                            w2b = P.tile([128, C], bf16, name=f"w2_{b}_{ch}_{ft}", tag="w2", bufs=4)
                            nc.gpsimd.dma_start(w2b[:], w2T[ft * 128:(ft + 1) * 128, :])
                            ph1 = ffp.tile([128, 512], f32, name=f"h1_{b}_{ch}_{ft}", tag="h1", bufs=2)
                            for k in range(KT):
                                nc.tensor.matmul(ph1[:], w1b[:, k * 128:(k + 1) * 128],
                                                 xl2[k][:, sl],
                                                 start=(k == 0), stop=(k == KT - 1))
                            gt = P.tile([128, 512], bf16, name=f"g_{b}_{ch}_{ft}", tag="g", bufs=3)
                            nc.scalar.activation(gt[:], ph1[:], Act.Gelu, bias=b1_sb[ft][:])
                            if prev is not None:
                                pw2, pgt, pft = prev
                                for mt in range(KT):
                                    nc.tensor.matmul(ph2[mt][:], pw2[:, mt * 128:(mt + 1) * 128],
                                                     pgt[:], start=(pft == 0), stop=(pft == FT - 1))
                            prev = (w2b, gt, ft)
                        pw2, pgt, pft = prev
                        for mt in range(KT):
                            nc.tensor.matmul(ph2[mt][:], pw2[:, mt * 128:(mt + 1) * 128],
                                             pgt[:], start=(pft == 0), stop=(pft == FT - 1))
                        for mt in range(KT):
                            ob = P.tile([128, 512], f32, name=f"o_{b}_{ch}_{mt}", tag="ob", bufs=3)
                            nc.vector.tensor_add(ob[:], x2b[mt][:, sl], ph2[mt][:])
                            nc.sync.dma_start(outT[b, mt * 128:(mt + 1) * 128, sl], ob[:])

    n_hoisted = _split_sync_waits(nc)
    print(f"_split_sync_waits: hoisted waits onto {n_hoisted} carrier instructions")
    return nc


def _host_prep(inputs):
    """Fold LN/BN affines into weights; build packed bf16 arrays."""
    f = lambda k: np.asarray(inputs[k], np.float32)
    bfc = lambda a: np.ascontiguousarray(a.astype(ml_dtypes.bfloat16))
    x = f("x")                         # (B, n, C)
    ln1_g, ln1_b = f("ln1_g"), f("ln1_b")
    ln2_g, ln2_b = f("ln2_g"), f("ln2_b")

    prep = {}
    xT = np.ascontiguousarray(x.transpose(0, 2, 1))   # (B, C, n)
    prep["xTb"] = bfc(xT)

    diag9 = {}
    badj = {}
    for nm in ["q", "k", "v"]:
        w = f(f"dw_w_{nm}")[:, 0]                     # (C,3,3)
        w_eff = w * ln1_g[:, None, None]
        cb = f(f"dw_b_{nm}") + ln1_b * w.sum((1, 2))  # exact only if ln1_b == 0 (boundary)
        assert np.abs(cb).max() < 1e-30, "nonzero conv bias not implemented on device"
        sc = f(f"bn_g_{nm}") / np.sqrt(f(f"bn_v_{nm}") + EPS)
        sh = f(f"bn_b_{nm}") - f(f"bn_m_{nm}") * sc
        W = f(f"W_{nm}")
        W_eff = W * sc[None, :]
        # device computes elu+1 (the -1 is folded here); also BN shift
        b_eff = f(f"b_{nm}") + W @ sh - W_eff.sum(1)
        # pack 9 taps of diagonal matrices: [KT, 128, 9*128]
        d = np.zeros((KT, 128, 9 * 128), np.float32)
        for kt in range(KT):
            ww = w_eff[kt * 128:(kt + 1) * 128]       # (128,3,3)
            for tap in range(9):
                dy, dx = tap // 3, tap % 3
                d[kt, np.arange(128), tap * 128 + np.arange(128)] = ww[:, dy, dx]
        diag9[nm] = bfc(d)
        badj[nm] = b_eff
        prep[f"w{nm}T"] = bfc(np.ascontiguousarray(W_eff.T))
    prep["dq9"], prep["dk9"], prep["dv9"] = diag9["q"], diag9["k"], diag9["v"]
    prep["bq"] = np.ascontiguousarray(badj["q"].reshape(KT, 128).T)
    prep["bk"] = np.ascontiguousarray(badj["k"].reshape(KT, 128).T)
    prep["bva"] = np.ascontiguousarray(badj["v"].reshape(KT, 128).T)

    W1 = f("W1") * ln2_g[None, :]                     # (FF, C)
    b1 = f("b1") + f("W1") @ ln2_b
    W2 = f("W2")                                      # (C, FF)
    assert np.abs(f("b2")).max() < 1e-30, "nonzero b2 not implemented on device"
    W1T = W1.T                                        # (C, FF) = [cin, f]
    w1pk = np.zeros((FT, 128, C), np.float32)         # [ft, cin_p, kt*128+f]
    for ft in range(FT):
        blk = W1T[:, ft * 128:(ft + 1) * 128]         # (C, 128)
        w1pk[ft] = blk.reshape(KT, 128, 128).transpose(1, 0, 2).reshape(128, C)
    prep["w1p"] = bfc(w1pk)
    prep["w2T"] = bfc(np.ascontiguousarray(W2.T))     # (FF, C)
    prep["b1"] = np.ascontiguousarray(b1.reshape(FT, 128).T)
    prep["ones_sq"] = np.ones((128, 128), ml_dtypes.bfloat16)
    return prep


def kernel(**inputs):
    from concourse.bass_utils import run_bass_kernel_spmd

    _patch_compiler(ldw_opt=_BUILD_CACHE.get("ldw_opt", False))
    if "nc" not in _BUILD_CACHE:
        _BUILD_CACHE["nc"] = _build_program()
    nc = _BUILD_CACHE["nc"]

    prep = _host_prep(inputs)
    shared = {k: v for k, v in prep.items() if k != "xTb"}
    in_maps = []
    for c in range(NCORES):
        im = dict(shared)
        im["xTb"] = np.ascontiguousarray(prep["xTb"][c * BL:(c + 1) * BL])
        in_maps.append(im)

    res = run_bass_kernel_spmd(nc, in_maps, list(range(NCORES)),
                               **_BUILD_CACHE.get("run_kwargs", {}))
    _BUILD_CACHE["last_results"] = res
    outs = [res.results[c]["outT"].transpose(0, 2, 1) for c in range(NCORES)]
    return np.ascontiguousarray(np.concatenate(outs, 0).astype(np.float32))
